# revision 8
# baseline (speedup 1.0000x reference)
"""Trainium2 Bass kernel for nn_AttentiveEncoderPOS (embed+concat+linear+self-attention).

Strategy (8 cores, SPMD, no collectives):
  - Each core receives input_ids/pos_ids ROTATED so that its 1024-row slice
    comes first. Softmax attention is invariant to key/value permutation, so
    each core computes the full L = concat(emb[ids], pos[pids]) @ W.T + b
    (keys/values, in its own order) and attends only its first 1024 rows
    (queries) against all 8192 keys. Output rows i*1024:(i+1)*1024 come from
    core i directly.
  - Layout: L is produced TRANSPOSED (L.T, h on partitions) by the linear
    matmul; scores are computed transposed (keys on partitions, q on free) so
    that exp(scores) feeds the A@V matmul directly as the stationary operand
    and the softmax denominator is a per-partition scale after A@V.
  - bf16 matmul inputs, fp32 PSUM accumulation. Scores are tiny (|s|<0.01)
    so exp() without max-subtraction is exact softmax.
"""

import numpy as np

import concourse.bass as bass
import concourse.mybir as mybir
from concourse import bacc
from concourse.tile import TileContext
from concourse.bass_utils import run_bass_kernel_spmd
from concourse.masks import make_identity

N = 8192
H = 1024
VOCAB = 50257
POS = 64
NCORES = 8
NL = N // NCORES          # 1024 query rows per core
P = 128
KT = N // P               # 64 key tiles
HT = H // P               # 8 h tiles
CHUNK = 512
NCH = N // CHUNK          # 16 phase-1 chunks
RT = CHUNK // P           # 4 row tiles / chunk
K2 = 2 * H
KTI = K2 // P             # 16 contraction tiles for the linear
QTN = NL // P             # 8 q tiles
BLK = 8                   # key tiles per phase-2 block (PSUM accum chain len)
NBLK = KT // BLK
SCALE = 1.0 / 32.0        # 1/sqrt(H)

BF = mybir.dt.bfloat16
F32 = mybir.dt.float32
I32 = mybir.dt.int32
EXP = mybir.ActivationFunctionType.Exp


def build_nc():
    nc = bacc.Bacc()
    ids = nc.declare_dram_parameter("ids", [KT, P, 1], I32, isOutput=False)
    pids = nc.declare_dram_parameter("pids", [KT, P, 1], I32, isOutput=False)
    emb = nc.declare_dram_parameter("emb", [VOCAB, H], F32, isOutput=False)
    pemb = nc.declare_dram_parameter("pemb", [POS, H], F32, isOutput=False)
    wt = nc.declare_dram_parameter("wt", [K2, H], F32, isOutput=False)  # W.T
    bias = nc.declare_dram_parameter("bias", [HT, P, 1], F32, isOutput=False)
    out = nc.declare_dram_parameter("out", [NL, H], F32, isOutput=True)

    # L.T tile-blocked: [key-tile][h-tile][128 h, 128 key] bf16
    lt_d = nc.dram_tensor("lt_d", [KT, HT, P, P], BF)
    # V (= L, natural layout): [key-tile][128 key, 1024 h] bf16
    v_d = nc.dram_tensor("v_d", [KT, P, H], BF)

    with TileContext(nc) as tc:
        with tc.tile_pool(name="const", bufs=1) as const:
            ident = const.tile([P, P], BF)
            make_identity(nc, ident[:])
            ones = const.tile([P, 1], BF)
            nc.gpsimd.memset(ones[:], 1.0)
            ident32 = const.tile([P, P], F32)
            make_identity(nc, ident32[:])
            b_row = const.tile([1, H], F32)
            nc.sync.dma_start(
                out=b_row[0:1, :], in_=bias.rearrange("h p u -> u (h p)")
            )
            b_sb = const.tile([P, HT], F32)
            for ht in range(HT):
                nc.sync.dma_start(out=b_sb[:, ht : ht + 1], in_=bias[ht])

            # ---------------- Phase 1: L.T and V production ----------------
            with (
                tc.tile_pool(name="wtp", bufs=KTI) as wtp,
                tc.tile_pool(name="wld", bufs=2) as wld,
                tc.tile_pool(name="idp", bufs=8) as idp,
                tc.tile_pool(name="xfp", bufs=3) as xfp,
                tc.tile_pool(name="xbp", bufs=3) as xbp,
                tc.tile_pool(name="xtp", bufs=2 * KTI) as xtp,
                tc.tile_pool(name="ltp", bufs=2 * HT) as ltp,
                tc.tile_pool(name="lup", bufs=2 * HT) as lup,
                tc.tile_pool(name="vp", bufs=8) as vp,
                tc.tile_pool(name="tps", bufs=4, space="PSUM") as tps,
                tc.tile_pool(name="mps", bufs=2, space="PSUM") as mps,
            ):
                # W.T -> bf16 SBUF, one [128, H] tile per contraction k-tile
                wtb = []
                for k in range(KTI):
                    wf = wld.tile([P, H], F32, tag="wld")
                    nc.sync.dma_start(out=wf[:], in_=wt[k * P : (k + 1) * P, :])
                    wb = wtp.tile([P, H], BF, tag="wtb")
                    nc.vector.tensor_copy(out=wb[:], in_=wf[:])
                    wtb.append(wb)

                for ch in range(NCH):
                    # gather + transpose X for this chunk of 512 rows
                    xts = []
                    for k in range(KTI):
                        xts.append(xtp.tile([P, CHUNK], BF, tag="xt", name="xt"))
                    for rt in range(RT):
                        t = ch * RT + rt
                        idt = idp.tile([P, 1], I32, tag="id")
                        nc.sync.dma_start(out=idt[:], in_=ids[t])
                        pidt = idp.tile([P, 1], I32, tag="pid")
                        nc.sync.dma_start(out=pidt[:], in_=pids[t])
                        xf = xfp.tile([P, K2], F32, tag="xf")
                        nc.gpsimd.indirect_dma_start(
                            out=xf[:, 0:H],
                            out_offset=None,
                            in_=emb[:],
                            in_offset=bass.IndirectOffsetOnAxis(ap=idt[:, :1], axis=0),
                        )
                        nc.gpsimd.indirect_dma_start(
                            out=xf[:, H:K2],
                            out_offset=None,
                            in_=pemb[:],
                            in_offset=bass.IndirectOffsetOnAxis(ap=pidt[:, :1], axis=0),
                        )
                        xb = xbp.tile([P, K2], BF, tag="xb")
                        nc.vector.tensor_copy(out=xb[:], in_=xf[:])
                        for k in range(KTI):
                            pt = tps.tile([P, P], BF, tag="tp")
                            nc.tensor.transpose(
                                pt[:], xb[:, k * P : (k + 1) * P], ident[:]
                            )
                            nc.vector.tensor_copy(
                                out=xts[k][:, rt * P : (rt + 1) * P], in_=pt[:]
                            )

                    # linear: L.T[ht, chunk] = sum_k W.T[k,ht].T @ X.T[k,chunk]
                    lts = []
                    for ht in range(HT):
                        pm = mps.tile([P, CHUNK], F32, tag="mp")
                        for k in range(KTI):
                            nc.tensor.matmul(
                                pm[:],
                                lhsT=wtb[k][:, ht * P : (ht + 1) * P],
                                rhs=xts[k][:],
                                start=(k == 0),
                                stop=(k == KTI - 1),
                            )
                        lt = ltp.tile([P, CHUNK], BF, tag="lt")
                        nc.vector.tensor_add(
                            out=lt[:],
                            in0=pm[:],
                            in1=b_sb[:, ht : ht + 1].to_broadcast([P, CHUNK]),
                        )
                        ltu = lup.tile([P, CHUNK], BF, tag="ltu")
                        nc.vector.tensor_copy(out=ltu[:], in_=pm[:])
                        lts.append(ltu)
                        for kb in range(RT):
                            nc.sync.dma_start(
                                out=lt_d[ch * RT + kb, ht],
                                in_=lt[:, kb * P : (kb + 1) * P],
                            )
                    # V tiles: transpose L.T chunk back to natural layout
                    for kb in range(RT):
                        vt = vp.tile([P, H], BF, tag="v")
                        for ht in range(HT):
                            pt = tps.tile([P, P], BF, tag="tp")
                            nc.tensor.transpose(
                                pt[:], lts[ht][:, kb * P : (kb + 1) * P], ident[:]
                            )
                            nc.vector.tensor_copy(
                                out=vt[:, ht * P : (ht + 1) * P], in_=pt[:]
                            )
                        nc.sync.dma_start(out=v_d[ch * RT + kb], in_=vt[:])

            # ---------------- Phase 2: attention ----------------
            with (
                tc.tile_pool(name="qtp", bufs=HT) as qtp,
                tc.tile_pool(name="op", bufs=QTN) as op,
                tc.tile_pool(name="lkp", bufs=4) as lkp,
                tc.tile_pool(name="ep", bufs=2 * BLK) as ep,
                tc.tile_pool(name="vp2", bufs=2 * BLK) as vp2,
                tc.tile_pool(name="fin", bufs=2) as fin,
                tc.tile_pool(name="sps", bufs=2, space="PSUM") as sps,
                tc.tile_pool(name="ops", bufs=2, space="PSUM") as ops,
                tc.tile_pool(name="cps", bufs=1, space="PSUM") as cps,
            ):
                # Q.T resident: first NL columns of L.T
                qts = []
                for ht in range(HT):
                    q = qtp.tile([P, NL], BF, tag="qt")
                    for k in range(QTN):
                        nc.sync.dma_start(
                            out=q[:, k * P : (k + 1) * P], in_=lt_d[k, ht]
                        )
                    qts.append(q)

                psum_c = cps.tile([P, QTN], F32, tag="cs")
                out_sb = []
                for qt in range(QTN):
                    out_sb.append(op.tile([P, H], F32, tag="o", name="o"))

                for blk in range(NBLK):
                    es = []
                    vts = []
                    for j in range(BLK):
                        kt = blk * BLK + j
                        ltk = lkp.tile([P, H], BF, tag="lk")
                        for ht in range(HT):
                            nc.sync.dma_start(
                                out=ltk[:, ht * P : (ht + 1) * P], in_=lt_d[kt, ht]
                            )
                        e = ep.tile([P, NL], BF, tag="e")
                        for qc in range(NL // CHUNK):
                            ps = sps.tile([P, CHUNK], F32, tag="sp")
                            for ht in range(HT):
                                nc.tensor.matmul(
                                    ps[:],
                                    lhsT=ltk[:, ht * P : (ht + 1) * P],
                                    rhs=qts[ht][:, qc * CHUNK : (qc + 1) * CHUNK],
                                    start=(ht == 0),
                                    stop=(ht == HT - 1),
                                )
                            nc.scalar.activation(
                                out=e[:, qc * CHUNK : (qc + 1) * CHUNK],
                                in_=ps[:],
                                func=EXP,
                                scale=SCALE,
                            )
                        es.append(e)
                        # colsum accumulation (denominator), one chain per q tile
                        for qt in range(QTN):
                            nc.tensor.matmul(
                                psum_c[:, qt : qt + 1],
                                lhsT=e[:, qt * P : (qt + 1) * P],
                                rhs=ones[:],
                                start=(kt == 0),
                                stop=(kt == KT - 1),
                            )
                        vt = vp2.tile([P, H], BF, tag="v2")
                        nc.sync.dma_start(out=vt[:], in_=v_d[kt])
                        vts.append(vt)

                    last_blk = blk == NBLK - 1
                    if last_blk:
                        # colsum is complete: build its row layout for the
                        # exact rank-1 bias term colsum[q] * b[h]
                        cs_sb = fin.tile([P, QTN], F32, tag="cs_sb")
                        nc.vector.tensor_copy(out=cs_sb[:], in_=psum_c[:])
                        cs_row = fin.tile([1, NL], F32, tag="cs_row")
                        for qt in range(QTN):
                            cs_tp = sps.tile([1, P], F32, tag="ct", bufs=1)
                            nc.tensor.transpose(
                                cs_tp[:], cs_sb[:, qt : qt + 1], ident32[:]
                            )
                            nc.vector.tensor_copy(
                                out=cs_row[0:1, qt * P : (qt + 1) * P], in_=cs_tp[:]
                            )
                    for qt in range(QTN):
                        po = ops.tile([P, H], F32, tag="op")
                        for j in range(BLK):
                            for hh in range(H // CHUNK):
                                nc.tensor.matmul(
                                    po[:, hh * CHUNK : (hh + 1) * CHUNK],
                                    lhsT=es[j][:, qt * P : (qt + 1) * P],
                                    rhs=vts[j][:, hh * CHUNK : (hh + 1) * CHUNK],
                                    start=(j == 0),
                                    stop=(j == BLK - 1 and not last_blk),
                                )
                        if last_blk:
                            for hh in range(H // CHUNK):
                                nc.tensor.matmul(
                                    po[:, hh * CHUNK : (hh + 1) * CHUNK],
                                    lhsT=cs_row[0:1, qt * P : (qt + 1) * P],
                                    rhs=b_row[0:1, hh * CHUNK : (hh + 1) * CHUNK],
                                    start=False,
                                    stop=True,
                                )
                        if blk == 0:
                            nc.vector.tensor_copy(out=out_sb[qt][:], in_=po[:])
                        else:
                            nc.vector.tensor_add(
                                out=out_sb[qt][:], in0=out_sb[qt][:], in1=po[:]
                            )

                rec = fin.tile([P, QTN], F32, tag="rec")
                nc.vector.reciprocal(rec[:], psum_c[:])
                for qt in range(QTN):
                    nc.vector.tensor_mul(
                        out=out_sb[qt][:],
                        in0=out_sb[qt][:],
                        in1=rec[:, qt : qt + 1].to_broadcast([P, H]),
                    )
                    nc.sync.dma_start(
                        out=out[qt * P : (qt + 1) * P, :], in_=out_sb[qt][:]
                    )
    nc.finalize()
    return nc


def _prep_inputs(inputs):
    ids = np.asarray(inputs["input_ids"]).astype(np.int32)
    pids = np.asarray(inputs["pos_ids"]).astype(np.int32)
    emb = np.asarray(inputs["emb"], dtype=np.float32)
    pemb = np.asarray(inputs["pos_emb"], dtype=np.float32)
    W = np.asarray(inputs["W"], dtype=np.float32)
    b = np.asarray(inputs["b"], dtype=np.float32)
    wt = np.ascontiguousarray(W.T)                      # [2H, H]
    bias = np.ascontiguousarray(b.reshape(HT, P, 1))
    in_maps = []
    for i in range(NCORES):
        r = np.roll(ids, -NL * i)
        rp = np.roll(pids, -NL * i)
        in_maps.append(
            {
                "ids": np.ascontiguousarray(r.reshape(KT, P, 1)),
                "pids": np.ascontiguousarray(rp.reshape(KT, P, 1)),
                "emb": emb,
                "pemb": pemb,
                "wt": wt,
                "bias": bias,
            }
        )
    return in_maps


def run(inputs, trace=False):
    nc = build_nc()
    in_maps = _prep_inputs(inputs)
    res = run_bass_kernel_spmd(nc, in_maps, list(range(NCORES)), trace=trace)
    out = np.concatenate([res.results[i]["out"] for i in range(NCORES)], axis=0)
    return out, res


def kernel(**inputs):
    out, _ = run(inputs, trace=False)
    return out


# revision 10
# speedup vs baseline: 8.7548x; 8.7548x over previous
"""Trainium2 Bass kernel for nn_AttentiveEncoderPOS (embed+concat+linear+self-attention).

Strategy (8 cores, SPMD, no collectives):
  - Each core receives input_ids/pos_ids ROTATED so that its 1024-row slice
    comes first. Softmax attention is invariant to key/value permutation, so
    each core computes the full L = concat(emb[ids], pos[pids]) @ W.T + b
    (keys/values, in its own order) and attends only its first 1024 rows
    (queries) against all 8192 keys. Output rows i*1024:(i+1)*1024 come from
    core i directly.
  - Layout: L is produced TRANSPOSED (L.T, h on partitions) by the linear
    matmul; scores are computed transposed (keys on partitions, q on free) so
    that exp(scores) feeds the A@V matmul directly as the stationary operand
    and the softmax denominator is a per-partition scale after A@V.
  - bf16 matmul inputs, fp32 PSUM accumulation. Scores are tiny (|s|<0.01)
    so exp() without max-subtraction is exact softmax.
"""

import numpy as np

import concourse.bass as bass
import concourse.mybir as mybir
from concourse import bacc
from concourse.tile import TileContext
from concourse.bass_utils import run_bass_kernel_spmd
from concourse.masks import make_identity

N = 8192
H = 1024
VOCAB = 50257
POS = 64
NCORES = 8
NL = N // NCORES          # 1024 query rows per core
P = 128
KT = N // P               # 64 key tiles
HT = H // P               # 8 h tiles
CHUNK = 512
NCH = N // CHUNK          # 16 phase-1 chunks
RT = CHUNK // P           # 4 row tiles / chunk
K2 = 2 * H
KTI = K2 // P             # 16 contraction tiles for the linear
QTN = NL // P             # 8 q tiles
BLK = 8                   # key tiles per phase-2 block (PSUM accum chain len)
NBLK = KT // BLK
SCALE = 1.0 / 32.0        # 1/sqrt(H)

BF = mybir.dt.bfloat16
F32 = mybir.dt.float32
I32 = mybir.dt.int32
EXP = mybir.ActivationFunctionType.Exp


def build_nc():
    nc = bacc.Bacc()
    ids = nc.declare_dram_parameter("ids", [KT, P, 1], I32, isOutput=False)
    pids = nc.declare_dram_parameter("pids", [KT, P, 1], I32, isOutput=False)
    emb = nc.declare_dram_parameter("emb", [VOCAB, H], F32, isOutput=False)
    pemb = nc.declare_dram_parameter("pemb", [POS, H], F32, isOutput=False)
    wt = nc.declare_dram_parameter("wt", [K2, H], F32, isOutput=False)  # W.T
    bias = nc.declare_dram_parameter("bias", [HT, P, 1], F32, isOutput=False)
    out = nc.declare_dram_parameter("out", [NL, H], F32, isOutput=True)

    # L.T tile-blocked: [key-tile][h-tile][128 h, 128 key] bf16
    lt_d = nc.dram_tensor("lt_d", [KT, HT, P, P], BF)
    # V (= L, natural layout): [key-tile][128 key, 1024 h] bf16
    v_d = nc.dram_tensor("v_d", [KT, P, H], BF)

    with TileContext(nc) as tc:
        with tc.tile_pool(name="const", bufs=1) as const:
            ident = const.tile([P, P], BF)
            make_identity(nc, ident[:])
            ones = const.tile([P, 1], BF)
            nc.gpsimd.memset(ones[:], 1.0)
            ident32 = const.tile([P, P], F32)
            make_identity(nc, ident32[:])
            b_row = const.tile([1, H], F32)
            nc.sync.dma_start(
                out=b_row[0:1, :], in_=bias.rearrange("h p u -> u (h p)")
            )
            b_sb = const.tile([P, HT], F32)
            nc.sync.dma_start(
                out=b_sb[:].rearrange("p (h u) -> p h u", h=HT),
                in_=bias.rearrange("h p u -> p h u"),
            )

            # ---------------- Phase 1: L.T and V production ----------------
            with (
                tc.tile_pool(name="wtp", bufs=KTI) as wtp,
                tc.tile_pool(name="wld", bufs=2) as wld,
                tc.tile_pool(name="idp", bufs=8) as idp,
                tc.tile_pool(name="xfp", bufs=3) as xfp,
                tc.tile_pool(name="xbp", bufs=RT + 2) as xbp,
                tc.tile_pool(name="xtp", bufs=2 * KTI) as xtp,
                tc.tile_pool(name="ltp", bufs=2 * HT) as ltp,
                tc.tile_pool(name="lup", bufs=2 * HT) as lup,
                tc.tile_pool(name="vp", bufs=8) as vp,
                tc.tile_pool(name="tps", bufs=3, space="PSUM") as tps,
                tc.tile_pool(name="mps", bufs=2, space="PSUM") as mps,
            ):
                # W.T -> bf16 SBUF, one [128, H] tile per contraction k-tile
                wtb = []
                for k in range(KTI):
                    wf = wld.tile([P, H], F32, tag="wld")
                    nc.sync.dma_start(out=wf[:], in_=wt[k * P : (k + 1) * P, :])
                    wb = wtp.tile([P, H], BF, tag="wtb")
                    nc.vector.tensor_copy(out=wb[:], in_=wf[:])
                    wtb.append(wb)

                for ch in range(NCH):
                    # gather + transpose X for this chunk of 512 rows
                    xts = []
                    for k in range(KTI):
                        xts.append(xtp.tile([P, CHUNK], BF, tag="xt", name="xt"))
                    xbs = []
                    for rt in range(RT):
                        t = ch * RT + rt
                        idt = idp.tile([P, 1], I32, tag="id")
                        nc.sync.dma_start(out=idt[:], in_=ids[t])
                        pidt = idp.tile([P, 1], I32, tag="pid")
                        nc.sync.dma_start(out=pidt[:], in_=pids[t])
                        xf = xfp.tile([P, K2], F32, tag="xf")
                        nc.gpsimd.indirect_dma_start(
                            out=xf[:, 0:H],
                            out_offset=None,
                            in_=emb[:],
                            in_offset=bass.IndirectOffsetOnAxis(ap=idt[:, :1], axis=0),
                        )
                        nc.gpsimd.indirect_dma_start(
                            out=xf[:, H:K2],
                            out_offset=None,
                            in_=pemb[:],
                            in_offset=bass.IndirectOffsetOnAxis(ap=pidt[:, :1], axis=0),
                        )
                        xb = xbp.tile([P, K2], BF, tag="xb")
                        nc.vector.tensor_copy(out=xb[:], in_=xf[:])
                        xbs.append(xb)
                    for k in range(KTI):
                        pt = tps.tile([P, CHUNK], BF, tag="tp")
                        for rt in range(RT):
                            nc.tensor.transpose(
                                pt[:, rt * P : (rt + 1) * P],
                                xbs[rt][:, k * P : (k + 1) * P],
                                ident[:],
                            )
                        nc.vector.tensor_copy(out=xts[k][:], in_=pt[:])

                    # linear: L.T[ht, chunk] = sum_k W.T[k,ht].T @ X.T[k,chunk]
                    lts = []
                    for ht in range(HT):
                        pm = mps.tile([P, CHUNK], F32, tag="mp")
                        for k in range(KTI):
                            nc.tensor.matmul(
                                pm[:],
                                lhsT=wtb[k][:, ht * P : (ht + 1) * P],
                                rhs=xts[k][:],
                                start=(k == 0),
                                stop=(k == KTI - 1),
                            )
                        lt = ltp.tile([P, CHUNK], BF, tag="lt")
                        nc.vector.tensor_add(
                            out=lt[:],
                            in0=pm[:],
                            in1=b_sb[:, ht : ht + 1].to_broadcast([P, CHUNK]),
                        )
                        ltu = lup.tile([P, CHUNK], BF, tag="ltu")
                        nc.vector.tensor_copy(out=ltu[:], in_=pm[:])
                        lts.append(ltu)
                        nc.sync.dma_start(
                            out=lt_d[ch * RT : (ch + 1) * RT, ht].rearrange(
                                "kb p c -> p kb c"
                            ),
                            in_=lt[:].rearrange("p (kb c) -> p kb c", kb=RT),
                        )
                    # V tiles: transpose L.T chunk back to natural layout
                    for kb in range(RT):
                        vt = vp.tile([P, H], BF, tag="v")
                        pt = tps.tile([P, H], BF, tag="tpv")
                        for ht in range(HT):
                            nc.tensor.transpose(
                                pt[:, ht * P : (ht + 1) * P],
                                lts[ht][:, kb * P : (kb + 1) * P],
                                ident[:],
                            )
                        nc.vector.tensor_copy(out=vt[:], in_=pt[:])
                        nc.sync.dma_start(out=v_d[ch * RT + kb], in_=vt[:])

            # ---------------- Phase 2: attention ----------------
            with (
                tc.tile_pool(name="qtp", bufs=HT) as qtp,
                tc.tile_pool(name="op", bufs=QTN) as op,
                tc.tile_pool(name="lkp", bufs=4) as lkp,
                tc.tile_pool(name="ep", bufs=2 * BLK) as ep,
                tc.tile_pool(name="vp2", bufs=2 * BLK) as vp2,
                tc.tile_pool(name="fin", bufs=2) as fin,
                tc.tile_pool(name="sps", bufs=2, space="PSUM") as sps,
                tc.tile_pool(name="ops", bufs=2, space="PSUM") as ops,
                tc.tile_pool(name="cps", bufs=1, space="PSUM") as cps,
            ):
                # Q.T resident: first NL columns of L.T
                qts = []
                for ht in range(HT):
                    q = qtp.tile([P, NL], BF, tag="qt")
                    nc.sync.dma_start(
                        out=q[:].rearrange("p (k c) -> p k c", k=QTN),
                        in_=lt_d[0:QTN, ht].rearrange("k p c -> p k c"),
                    )
                    qts.append(q)

                psum_c = cps.tile([P, QTN], F32, tag="cs")
                out_sb = []
                for qt in range(QTN):
                    out_sb.append(op.tile([P, H], F32, tag="o", name="o"))

                for blk in range(NBLK):
                    es = []
                    vts = []
                    for j in range(BLK):
                        kt = blk * BLK + j
                        ltk = lkp.tile([P, H], BF, tag="lk")
                        nc.sync.dma_start(
                            out=ltk[:].rearrange("p (h c) -> p h c", h=HT),
                            in_=lt_d[kt].rearrange("h p c -> p h c"),
                        )
                        e = ep.tile([P, NL], BF, tag="e")
                        for qc in range(NL // CHUNK):
                            ps = sps.tile([P, CHUNK], F32, tag="sp")
                            for ht in range(HT):
                                nc.tensor.matmul(
                                    ps[:],
                                    lhsT=ltk[:, ht * P : (ht + 1) * P],
                                    rhs=qts[ht][:, qc * CHUNK : (qc + 1) * CHUNK],
                                    start=(ht == 0),
                                    stop=(ht == HT - 1),
                                )
                            nc.scalar.activation(
                                out=e[:, qc * CHUNK : (qc + 1) * CHUNK],
                                in_=ps[:],
                                func=EXP,
                                scale=SCALE,
                            )
                        es.append(e)
                        # colsum accumulation (denominator), one chain per q tile
                        for qt in range(QTN):
                            nc.tensor.matmul(
                                psum_c[:, qt : qt + 1],
                                lhsT=e[:, qt * P : (qt + 1) * P],
                                rhs=ones[:],
                                start=(kt == 0),
                                stop=(kt == KT - 1),
                            )
                        vt = vp2.tile([P, H], BF, tag="v2")
                        nc.sync.dma_start(out=vt[:], in_=v_d[kt])
                        vts.append(vt)

                    last_blk = blk == NBLK - 1
                    if last_blk:
                        # colsum is complete: build its row layout for the
                        # exact rank-1 bias term colsum[q] * b[h]
                        cs_sb = fin.tile([P, QTN], F32, tag="cs_sb")
                        nc.vector.tensor_copy(out=cs_sb[:], in_=psum_c[:])
                        cs_row = fin.tile([1, NL], F32, tag="cs_row")
                        for qt in range(QTN):
                            cs_tp = sps.tile([1, P], F32, tag="ct", bufs=1)
                            nc.tensor.transpose(
                                cs_tp[:], cs_sb[:, qt : qt + 1], ident32[:]
                            )
                            nc.vector.tensor_copy(
                                out=cs_row[0:1, qt * P : (qt + 1) * P], in_=cs_tp[:]
                            )
                    for qt in range(QTN):
                        po = ops.tile([P, H], F32, tag="op")
                        for j in range(BLK):
                            for hh in range(H // CHUNK):
                                nc.tensor.matmul(
                                    po[:, hh * CHUNK : (hh + 1) * CHUNK],
                                    lhsT=es[j][:, qt * P : (qt + 1) * P],
                                    rhs=vts[j][:, hh * CHUNK : (hh + 1) * CHUNK],
                                    start=(j == 0),
                                    stop=(j == BLK - 1 and not last_blk),
                                )
                        if last_blk:
                            for hh in range(H // CHUNK):
                                nc.tensor.matmul(
                                    po[:, hh * CHUNK : (hh + 1) * CHUNK],
                                    lhsT=cs_row[0:1, qt * P : (qt + 1) * P],
                                    rhs=b_row[0:1, hh * CHUNK : (hh + 1) * CHUNK],
                                    start=False,
                                    stop=True,
                                )
                        if blk == 0:
                            nc.vector.tensor_copy(out=out_sb[qt][:], in_=po[:])
                        else:
                            nc.vector.tensor_add(
                                out=out_sb[qt][:], in0=out_sb[qt][:], in1=po[:]
                            )

                rec = fin.tile([P, QTN], F32, tag="rec")
                nc.vector.reciprocal(rec[:], psum_c[:])
                for qt in range(QTN):
                    nc.vector.tensor_mul(
                        out=out_sb[qt][:],
                        in0=out_sb[qt][:],
                        in1=rec[:, qt : qt + 1].to_broadcast([P, H]),
                    )
                    nc.sync.dma_start(
                        out=out[qt * P : (qt + 1) * P, :], in_=out_sb[qt][:]
                    )
    nc.finalize()
    return nc


def _prep_inputs(inputs):
    ids = np.asarray(inputs["input_ids"]).astype(np.int32)
    pids = np.asarray(inputs["pos_ids"]).astype(np.int32)
    emb = np.asarray(inputs["emb"], dtype=np.float32)
    pemb = np.asarray(inputs["pos_emb"], dtype=np.float32)
    W = np.asarray(inputs["W"], dtype=np.float32)
    b = np.asarray(inputs["b"], dtype=np.float32)
    wt = np.ascontiguousarray(W.T)                      # [2H, H]
    bias = np.ascontiguousarray(b.reshape(HT, P, 1))
    in_maps = []
    for i in range(NCORES):
        r = np.roll(ids, -NL * i)
        rp = np.roll(pids, -NL * i)
        in_maps.append(
            {
                "ids": np.ascontiguousarray(r.reshape(KT, P, 1)),
                "pids": np.ascontiguousarray(rp.reshape(KT, P, 1)),
                "emb": emb,
                "pemb": pemb,
                "wt": wt,
                "bias": bias,
            }
        )
    return in_maps


def run(inputs, trace=False):
    nc = build_nc()
    in_maps = _prep_inputs(inputs)
    res = run_bass_kernel_spmd(nc, in_maps, list(range(NCORES)), trace=trace)
    out = np.concatenate([res.results[i]["out"] for i in range(NCORES)], axis=0)
    return out, res


def kernel(**inputs):
    out, _ = run(inputs, trace=False)
    return out
